# revision 20
# baseline (speedup 1.0000x reference)
"""BiTreeLSTM (ChildSum bottom-up + Chain top-down) over a complete binary tree,
depth 14 (16383 nodes), on 8 Trainium2 NeuronCores.

Sharding: per-level contiguous node sharding. Core k owns, for every level
l >= 3, the k-th contiguous 1/8 slice of that level's nodes. Children of core
k's nodes at level l are exactly core k's nodes at level l+1, so both
recursions are communication-free except the top 3 levels (7 nodes), which
every core computes redundantly from (a) replicated input rows 0..6 and (b)
one tiny AllGather of level-3 (h, c).

Compute layout: everything transposed ([feature, node]). The child-sum deep
levels (11-13, 87% of the tree) run their x-projections and deep h-recurrence
as fp8e4 DoubleRow matmuls (2 contraction rows/cycle); quantization error
injected at the leaves attenuates through the child-sum recursion (measured
~4e-3 end to end vs the 2e-2 gate). The chain tree is error-sensitive at the
deep end (global max over node hiddens picks outliers), so phase B stays fp16
throughout. fp8 weights are pre-scaled by 512 so uniform(-1/32,1/32) entries
sit in e4m3's normal range; every consumer descales via a free activation
scale or a fused scalar_tensor_tensor multiply.
"""

import numpy as np
import ml_dtypes

import concourse.bass as bass
import concourse.mybir as mybir
import concourse.tile as tile
from concourse import bacc
from concourse.bass_utils import run_bass_kernel_spmd

AFT = mybir.ActivationFunctionType
ALU = mybir.AluOpType
DR = mybir.MatmulPerfMode.DoubleRow
H = mybir.dt.float16
H8 = mybir.dt.float8e4
F32 = mybir.dt.float32

DEPTH = 14
IN = 1024
MEM = 512
NCORES = 8
KCX = IN // 128   # 8 contraction chunks for fp16 x projections
KC8 = IN // 256   # 4 DoubleRow chunks for fp8 x projections
KCH = MEM // 128  # 4 contraction chunks for fp16 h projections
KH8 = MEM // 256  # 2 DoubleRow chunks for fp8 h projections
SW = 512.0        # fp8 weight pre-scale
ISW = 1.0 / SW

# per-core column layout: cols 0..6 = global nodes 0..6 (replicated);
# then levels 3..13 contiguously (core-local slices)
L_OFF = {}
_off = 7
for _l in range(3, DEPTH):
    L_OFF[_l] = _off
    _off += 2 ** (_l - 3)
NCOLS = _off          # 2054
SH_COLS = L_OFF[11]   # 262: top7 + levels 3..10
ND8 = NCOLS - SH_COLS  # 1792 deep cols (levels 11-13), fp8 copy

# phase-A gate-chunk order j: i(0..3) o(4..7) u(8..11) f(12..15);
# wxa (ifoux) block layout is i,f,o,u -> block index for each j:
WXA_BLK = [0, 1, 2, 3, 8, 9, 10, 11, 12, 13, 14, 15, 4, 5, 6, 7]

_PROG = None


def _bcast2(ap):
    """View [P, ..., N] as [P, ..., N, 2] with step 0 (each element twice)."""
    return bass.AP(tensor=ap.tensor, offset=ap.offset, ap=ap.ap + [[0, 2]])


def _pairs(ap):
    """(even, odd) views of the last dim interpreted as [..., t, 2]."""
    nd = len(ap.shape)
    letters = [chr(ord("a") + i) for i in range(nd - 1)]
    spec = " ".join(letters) + " (t two) -> " + " ".join(letters) + " t two"
    v = ap.rearrange(spec, two=2)
    idx = (slice(None),) * nd
    return v[idx + (0,)], v[idx + (1,)]


def _drv(ap):
    """View [P, 2d, ...] as [P, d, 2, ...]: DoubleRow pair view of a
    kc-chunked tensor (kc = d*2 + i)."""
    nd = len(ap.shape)
    letters = [chr(ord("a") + k) for k in range(nd - 2)]
    tail = " ".join(letters)
    spec = f"p (d i) {tail} -> p d i {tail}"
    return ap.rearrange(spec, i=2)


def build():
    """Build + compile the SPMD Bass program. Returns the Bacc object."""
    nc = bacc.Bacc("TRN2", target_bir_lowering=False, debug=False,
                   num_devices=NCORES)

    xT = nc.dram_tensor("xT", [IN, NCOLS], H, kind="ExternalInput")
    xT8 = nc.dram_tensor("xT8", [128, KC8 * 2 * ND8], H8, kind="ExternalInput")
    wxa = nc.dram_tensor("wxa", [IN, 4 * MEM], H, kind="ExternalInput")
    wxa8d = nc.dram_tensor("wxa8", [128, KC8 * 2 * 4 * MEM], H8,
                           kind="ExternalInput")
    wha = nc.dram_tensor("wha", [MEM, 3 * MEM], H, kind="ExternalInput")
    wha8d = nc.dram_tensor("wha8", [128, KH8 * 2 * 3 * MEM], H8,
                           kind="ExternalInput")
    wfh = nc.dram_tensor("wfh", [MEM, MEM], H, kind="ExternalInput")
    wfh8d = nc.dram_tensor("wfh8", [128, KH8 * 2 * MEM], H8,
                           kind="ExternalInput")
    wxb = nc.dram_tensor("wxb", [IN, 4 * MEM], H, kind="ExternalInput")
    whb = nc.dram_tensor("whb", [MEM, 4 * MEM], H, kind="ExternalInput")
    whb8d = nc.dram_tensor("whb8", [128, KH8 * 2 * 4 * MEM], H8,
                           kind="ExternalInput")
    ba = nc.dram_tensor("ba", [128, 16], F32, kind="ExternalInput")
    bb = nc.dram_tensor("bb", [128, 16], F32, kind="ExternalInput")
    sel3 = nc.dram_tensor("sel3", [128, 4], F32, kind="ExternalInput")
    out = nc.dram_tensor("out", [1, 2 * MEM], F32, kind="ExternalOutput")

    cc_in = nc.dram_tensor("cc_in", [2 * MEM], F32)
    cc_out = nc.dram_tensor("cc_out", [NCORES, 2 * MEM], F32,
                            addr_space="Shared")

    def xT_view(c0, w):
        return xT.ap()[:, c0:c0 + w].rearrange("(kc p) n -> p kc n", p=128)

    def xT8_view(c0, w):
        o = c0 - SH_COLS
        v = xT8.ap().rearrange("p (q i n) -> p q i n", q=KC8, i=2)
        return v[:, :, :, o:o + w]

    pool_stack = []

    with tile.TileContext(nc) as tc:

        def open_pool(name, bufs=1, space="SBUF"):
            cm = tc.tile_pool(name=name, bufs=bufs, space=space)
            p = cm.__enter__()
            pool_stack.append((name, cm))
            return p

        def close_pool(name):
            n, cm = pool_stack.pop()
            assert n == name, f"pool close order: expected {n}, got {name}"
            cm.__exit__(None, None, None)

        persist = open_pool("persist")
        pp4 = open_pool("pp4", bufs=6, space="PSUM")
        pp2 = open_pool("pp2", bufs=2, space="PSUM")

        ba_sb = persist.tile([128, 16], F32, tag="ba")
        bb_sb = persist.tile([128, 16], F32, tag="bb")
        sel3_sb = persist.tile([128, 4], F32, tag="sel3")
        cmax = persist.tile([128, 4], F32, tag="cmax")
        wx_t = persist.tile([128, KCX, 4 * MEM], H, tag="wx", name="wx")
        wha_sb = persist.tile([128, KCH, 3 * MEM], H, tag="wha")
        wfh_sb = persist.tile([128, KCH, MEM], H, tag="wfh")
        whb8_sb = persist.tile([128, KCH, 4 * MEM], H8, tag="whb8")
        xgtop = persist.tile([128, 16, 7], F32, tag="xgtop")
        h3all = persist.tile([128, 8, KCH], H, tag="h3all")
        c3all = persist.tile([128, 8, KCH], F32, tag="c3all")
        h3f = persist.tile([128, KCH, 1], F32, tag="h3f")
        c3f = persist.tile([128, KCH, 1], F32, tag="c3f")
        h3f32 = persist.tile([128, 8, KCH], F32, tag="h3f32")
        xTsh = persist.tile([128, KCX, SH_COLS], H, tag="xTsh")
        xgshB = persist.tile([128, 16, SH_COLS], H, tag="xgshB")

        # fp8 A-deep weight tiles live in the pAdeep pool (declared there;
        # dead once the deep child-sum levels finish, freeing their SBUF)
        fp8w = {}

        def wxa8_v():
            return _drv(fp8w["wxa8"][:])  # [128, q, i, 2048]

        # ============ helpers ============

        def alloc_hc(pool, M, with_c=True, hdt=H):
            Mp = max(M, 2)  # matmul moving dim must be >= 2; pad tiny levels
            h = pool.tile([128, KCH, Mp], hdt, tag=f"h{M}{hdt}", bufs=1,
                          name=f"h{M}")
            c = (pool.tile([128, KCH, Mp], F32, tag=f"c{M}", bufs=1,
                           name=f"c{M}") if with_c else None)
            # pad columns are never read (their matmul psum outputs are
            # never consumed), so they stay uninitialized
            return h, c

        def csum_cell(pool, W, xt8, hs8, h_pv8, c_pv, cb, h_dst,
                      c_dst, d0, leaf=False, emit_cb=None):
            """Child-sum LSTM cell for W nodes at dst offset d0.
            All matmuls fp8 DoubleRow; xt8 [128,q,i,W]; hs8/h_pv8 fp8
            [128, KCH, *] tiles (DR pair-viewed); c_pv fp32."""
            hs_v = _drv(hs8[:]) if hs8 is not None else None
            hp_v = _drv(h_pv8[:]) if h_pv8 is not None else None
            for mc in range(4):
                gates = {}
                for gi, j in (("i", mc), ("o", 4 + mc), ("u", 8 + mc)):
                    ps = pp4.tile([128, W], F32, tag="ps_g", name="psg")
                    blk = WXA_BLK[j]
                    for q in range(KC8):
                        nc.tensor.matmul(
                            ps[:], fp8w["wxa8v"][:, q, :, blk * 128:(blk + 1) * 128],
                            xt8[:, q, :, :], start=q == 0,
                            stop=(leaf and q == KC8 - 1), perf_mode=DR)
                    if not leaf:
                        for d in range(KH8):
                            nc.tensor.matmul(
                                ps[:], fp8w["wha8v"][:, d, :, j * 128:(j + 1) * 128],
                                hs_v[:, d, :, 0:W], start=False,
                                stop=d == KH8 - 1, perf_mode=DR)
                    g = pool.tile([128, W], H, tag=f"g_{gi}", name="g")
                    nc.scalar.activation(
                        out=g[:], in_=ps[:],
                        func=AFT.Tanh if gi == "u" else AFT.Sigmoid,
                        bias=ba_sb[:, j:j + 1], scale=ISW)
                    gates[gi] = g
                c_sl = c_dst[:, mc, d0:d0 + W]
                h_sl = h_dst[:, mc, d0:d0 + W]
                if leaf:
                    nc.vector.tensor_mul(c_sl, gates["i"][:], gates["u"][:])
                else:
                    psx = pp4.tile([128, W], F32, tag="ps_g", name="psx")
                    blk = WXA_BLK[12 + mc]
                    for q in range(KC8):
                        nc.tensor.matmul(
                            psx[:], fp8w["wxa8v"][:, q, :, blk * 128:(blk + 1) * 128],
                            xt8[:, q, :, :], start=q == 0, stop=q == KC8 - 1,
                            perf_mode=DR)
                    fx = pool.tile([128, W], F32, tag="fx", bufs=1, name="fx")
                    nc.scalar.activation(out=fx[:], in_=psx[:],
                                         func=AFT.Identity,
                                         bias=ba_sb[:, 12 + mc:13 + mc],
                                         scale=ISW)
                    psF = pp2.tile([128, 2 * W], F32, tag="ps_F", name="psF")
                    for d in range(KH8):
                        nc.tensor.matmul(
                            psF[:], fp8w["wfh8v"][:, d, :, mc * 128:(mc + 1) * 128],
                            hp_v[:, d, :, cb:cb + 2 * W], start=d == 0,
                            stop=d == KH8 - 1, perf_mode=DR)
                    f_sb = pool.tile([128, 2 * W], H, tag="f", bufs=1,
                                     name="f")
                    nc.vector.scalar_tensor_tensor(
                        f_sb[:].rearrange("p (t two) -> p t two", two=2),
                        psF[:].rearrange("p (t two) -> p t two", two=2),
                        ISW, _bcast2(fx[:]), op0=ALU.mult, op1=ALU.add)
                    nc.scalar.activation(out=f_sb[:], in_=f_sb[:],
                                         func=AFT.Sigmoid)
                    nc.vector.tensor_mul(c_sl, gates["i"][:], gates["u"][:])
                    f_e, f_o = _pairs(f_sb[:])
                    c_e, c_o = _pairs(c_pv[:, mc, cb:cb + 2 * W])
                    t1 = pool.tile([128, W], H, tag="t1", bufs=1, name="t1")
                    nc.vector.tensor_mul(t1[:], f_e, c_e)
                    nc.vector.tensor_add(c_sl, c_sl, t1[:])
                    t2 = pool.tile([128, W], H, tag="t2", bufs=1, name="t2")
                    nc.vector.tensor_mul(t2[:], f_o, c_o)
                    nc.vector.tensor_add(c_sl, c_sl, t2[:])
                th = pool.tile([128, W], H, tag="th", bufs=1, name="th")
                nc.scalar.activation(out=th[:], in_=c_sl, func=AFT.Tanh)
                nc.vector.tensor_mul(h_sl, gates["o"][:], th[:])
                if emit_cb is not None and mc in (1, 3):
                    emit_cb()

        def phaseA_level(pool, hc_pool, M, xg_off, xg_t, h_pv, c_pv,
                         wha_t, wfh_t):
            """Shallow child-sum level from precomputed x pre-activations.
            fp16 weights/h (unscaled)."""
            h_cur, c_cur = alloc_hc(hc_pool, M)
            Mp = max(M, 2)
            hs = pool.tile([128, KCH, Mp], H, tag="hsumS", bufs=1, name="hs")
            he, ho = _pairs(h_pv[:, :, 0:2 * M])
            nc.vector.tensor_add(hs[:, :, 0:M], he, ho)
            gate = pool.tile([128, 12, M], H, tag="gateA", bufs=1,
                             name="gate")
            gsz = max(1, 512 // Mp)
            for g0 in range(0, 12, gsz):
                g1 = min(12, g0 + gsz)
                ps = pp4.tile([128, g1 - g0, Mp], F32, tag="ps_g", name="psg")
                for j in range(g0, g1):
                    for kc in range(KCH):
                        nc.tensor.matmul(
                            ps[:, j - g0, :],
                            wha_t[:, kc, j * 128:(j + 1) * 128],
                            hs[:, kc, :], start=kc == 0, stop=kc == KCH - 1)
                pre = pool.tile([128, g1 - g0, M], F32, tag="preA", name="pre")
                nc.vector.tensor_add(pre[:], xg_t[:, g0:g1, xg_off:xg_off + M],
                                     ps[:, :, 0:M])
                if g0 < 8:
                    s1 = min(g1, 8)
                    nc.scalar.activation(out=gate[:, g0:s1, :],
                                         in_=pre[:, 0:s1 - g0, :],
                                         func=AFT.Sigmoid)
                if g1 > 8:
                    s0 = max(g0, 8)
                    nc.scalar.activation(out=gate[:, s0:g1, :],
                                         in_=pre[:, s0 - g0:g1 - g0, :],
                                         func=AFT.Tanh)
            fga = pool.tile([128, 4, 2 * M], H, tag="fgA", bufs=1,
                            name="fga")
            gF = max(1, 512 // (2 * M))
            for m0 in range(0, 4, gF):
                m1 = min(4, m0 + gF)
                psF = pp2.tile([128, m1 - m0, 2 * M], F32, tag="ps_F",
                               name="psF")
                for mc in range(m0, m1):
                    for kc in range(KCH):
                        nc.tensor.matmul(
                            psF[:, mc - m0, :],
                            wfh_t[:, kc, mc * 128:(mc + 1) * 128],
                            h_pv[:, kc, 0:2 * M], start=kc == 0,
                            stop=kc == KCH - 1)
                fxv = xg_t[:, 12 + m0:12 + m1, xg_off:xg_off + M]
                nc.vector.tensor_add(
                    fga[:, m0:m1, :].rearrange("p m (t two) -> p m t two",
                                               two=2),
                    psF[:].rearrange("p m (t two) -> p m t two", two=2),
                    _bcast2(fxv))
            nc.scalar.activation(out=fga[:], in_=fga[:], func=AFT.Sigmoid)
            fe, fo = _pairs(fga[:])
            ce, co = _pairs(c_pv[:, :, 0:2 * M])
            c_sl = c_cur[:, :, 0:M]
            h_sl = h_cur[:, :, 0:M]
            nc.vector.tensor_mul(c_sl, gate[:, 0:4, :], gate[:, 8:12, :])
            t1 = pool.tile([128, 4, M], H, tag="t1A", bufs=1, name="t1")
            nc.vector.tensor_mul(t1[:], fe, ce)
            nc.vector.tensor_add(c_sl, c_sl, t1[:])
            t2 = pool.tile([128, 4, M], H, tag="t2A", bufs=1, name="t2")
            nc.vector.tensor_mul(t2[:], fo, co)
            nc.vector.tensor_add(c_sl, c_sl, t2[:])
            th = pool.tile([128, 4, M], H, tag="thA", bufs=1, name="th")
            nc.scalar.activation(out=th[:], in_=c_sl, func=AFT.Tanh)
            nc.vector.tensor_mul(h_sl, gate[:, 4:8, :], th[:])
            return h_cur, c_cur

        # static pool for A-top transients: lets the top levels run
        # concurrently with the tail right after the collective lands
        pT = open_pool("pTop", bufs=2)
        hcT = open_pool("hcT")

        # ============ phase A ============

        def chain_update(pool, h_ap, M, mc=None):
            if mc is not None:  # single mem-chunk slab [128, M]
                rm = pool.tile([128, 1], F32, tag="rmax", bufs=1, name="rm")
                nc.vector.tensor_reduce(out=rm[:], in_=h_ap,
                                        axis=mybir.AxisListType.X, op=ALU.max)
                nc.vector.tensor_max(cmax[:, mc:mc + 1], cmax[:, mc:mc + 1],
                                     rm[:])
            elif M == 1:
                rm1 = pool.tile([128, KCH], F32, tag="rmaxq", bufs=1,
                                name="rm1")
                nc.vector.tensor_copy(out=rm1[:], in_=h_ap[:, :, 0])
                nc.vector.tensor_max(cmax[:], cmax[:], rm1[:])
            else:
                rm = pool.tile([128, KCH], F32, tag="rmaxq", bufs=1, name="rm")
                nc.vector.tensor_reduce(out=rm[:], in_=h_ap,
                                        axis=mybir.AxisListType.X, op=ALU.max)
                nc.vector.tensor_max(cmax[:], cmax[:], rm[:])

        def phaseB_level(pool, M, xg_off, h_pv, c_pv, direct=False,
                         root=False):
            """Chain level (fp16 throughout). Returns (h_cur, c_cur)."""
            h_cur, c_cur = alloc_hc(hcB, M)
            Pn = M if direct else M // 2
            Pp = max(Pn, 2)
            gate = pool.tile([128, 16, M], H, tag="gateB", bufs=1,
                             name="gate")
            if root:
                nc.scalar.activation(out=gate[:, 0:12, :],
                                     in_=xgshB[:, 0:12, xg_off:xg_off + M],
                                     func=AFT.Sigmoid)
                nc.scalar.activation(out=gate[:, 12:16, :],
                                     in_=xgshB[:, 12:16, xg_off:xg_off + M],
                                     func=AFT.Tanh)
            else:
                pre = pool.tile([128, 16, M], F32, tag="preB", bufs=1,
                                name="pre")
                gsz = max(1, 512 // Pp)
                for g0 in range(0, 16, gsz):
                    g1 = min(16, g0 + gsz)
                    ps = pp4.tile([128, g1 - g0, Pp], F32, tag="ps_g",
                                  name="psg")
                    for j in range(g0, g1):
                        for kc in range(KCH):
                            nc.tensor.matmul(
                                ps[:, j - g0, :],
                                whb_sb[:, kc, j * 128:(j + 1) * 128],
                                h_pv[:, kc, 0:Pp], start=kc == 0,
                                stop=kc == KCH - 1)
                    xgv = xgshB[:, g0:g1, xg_off:xg_off + M]
                    psv = ps[:, :, 0:Pn]
                    if direct:
                        nc.vector.tensor_add(pre[:, g0:g1, :], xgv, psv)
                    else:
                        nc.vector.tensor_add(
                            pre[:, g0:g1, :].rearrange(
                                "p q (t two) -> p q t two", two=2),
                            xgv.rearrange("p q (t two) -> p q t two", two=2),
                            _bcast2(psv))
                nc.scalar.activation(out=gate[:, 0:12, :], in_=pre[:, 0:12, :],
                                     func=AFT.Sigmoid)
                nc.scalar.activation(out=gate[:, 12:16, :],
                                     in_=pre[:, 12:16, :], func=AFT.Tanh)
            c_sl = c_cur[:, :, 0:M]
            h_sl = h_cur[:, :, 0:M]
            nc.vector.tensor_mul(c_sl, gate[:, 0:4, :], gate[:, 12:16, :])
            if not root:
                if direct:
                    t1 = pool.tile([128, 4, M], H, tag="t1B", bufs=1,
                                   name="t1")
                    nc.vector.tensor_mul(t1[:], gate[:, 8:12, :],
                                         c_pv[:, :, 0:Pn])
                    nc.vector.tensor_add(c_sl, c_sl, t1[:])
                else:
                    fe, fo = _pairs(gate[:, 8:12, :])
                    ce, co = _pairs(c_sl)
                    t1 = pool.tile([128, 4, Pn], H, tag="t1B", bufs=1,
                                   name="t1")
                    nc.vector.tensor_mul(t1[:], fe, c_pv[:, :, 0:Pn])
                    nc.vector.tensor_add(ce, ce, t1[:])
                    t2 = pool.tile([128, 4, Pn], H, tag="t2B", bufs=1,
                                   name="t2")
                    nc.vector.tensor_mul(t2[:], fo, c_pv[:, :, 0:Pn])
                    nc.vector.tensor_add(co, co, t2[:])
            th = pool.tile([128, 4, M], H, tag="thB", bufs=1, name="th")
            nc.scalar.activation(out=th[:], in_=c_sl, func=AFT.Tanh)
            nc.vector.tensor_mul(h_sl, gate[:, 4:8, :], th[:])
            chain_update(pool, h_cur[:, :, 0:M], M)
            return h_cur, c_cur

        def emit_b3_parent(h_pv, c_pv):
            hpar = pBs.tile([128, KCH, 2], H, tag="hpar", bufs=1)
            cpar = pBs.tile([128, KCH, 2], F32, tag="cpar", bufs=1)
            selv = sel3_sb[:]
            sel_b = bass.AP(tensor=selv.tensor, offset=selv.offset,
                            ap=[selv.ap[0], [0, KCH], selv.ap[1]])
            tm = pBs.tile([128, KCH, 4], F32, tag="selt", bufs=1)
            nc.vector.tensor_mul(tm[:], h_pv[:, :, 0:4], sel_b)
            with nc.allow_low_precision(reason="one-hot selection sum"):
                nc.vector.tensor_reduce(out=hpar[:, :, 0:1], in_=tm[:],
                                        axis=mybir.AxisListType.X, op=ALU.add)
            tc_ = pBs.tile([128, KCH, 4], F32, tag="selt", bufs=1)
            nc.vector.tensor_mul(tc_[:], c_pv[:, :, 0:4], sel_b)
            nc.vector.tensor_reduce(out=cpar[:, :, 0:1], in_=tc_[:],
                                    axis=mybir.AxisListType.X, op=ALU.add)
            return hpar, cpar

        hcB = open_pool("hcB")
        pBs = open_pool("pBsh", bufs=2)
        hcA = open_pool("hcA")
        pAd = open_pool("pAdeep", bufs=2)
        wxa8_sb = pAd.tile([128, KC8 * 2, 4 * MEM], H8, tag="wxa8", bufs=1)
        wha8_sb = pAd.tile([128, KH8 * 2, 3 * MEM], H8, tag="wha8", bufs=1)
        wfh8_sb = pAd.tile([128, KH8 * 2, MEM], H8, tag="wfh8", bufs=1)
        whb_sb = pAd.tile([128, KCH, 4 * MEM], H, tag="whb", bufs=1)
        fp8w["wxa8"] = wxa8_sb
        fp8w["wxa8v"] = _drv(wxa8_sb[:])
        fp8w["wha8v"] = _drv(wha8_sb[:])
        fp8w["wfh8v"] = _drv(wfh8_sb[:])
        whb8_v = _drv(whb8_sb[:])

        # wxb loads FIRST: the phase-B xg batch runs before phase A so the
        # whole B shallow chain hides under A-deep; fp8 A weights stream in
        # behind. Per-quarter DMAs keep the sync issue queue short.
        nc.sync.dma_start(out=ba_sb[:], in_=ba.ap())
        nc.sync.dma_start(out=bb_sb[:], in_=bb.ap())
        nc.sync.dma_start(out=sel3_sb[:], in_=sel3.ap())
        nc.vector.memset(cmax[:], -3.0e38)
        xt_pre = []
        for g in range(2):
            xt = pAd.tile([128, KC8, 2, 256], H8, tag="xt", bufs=2,
                          name="xt")
            nc.sync.dma_start(out=xt[:], in_=xT8_view(L_OFF[13] + g * 256,
                                                      256))
            xt_pre.append(xt)
        # fp8 A weights in leaf consumption order (i,o,u; f later)
        wxa8_r = wxa8d.ap().rearrange("p (q i m) -> p (q i) m", q=KC8, i=2)
        for g0 in (0, 1024, 1536):
            nc.sync.dma_start(out=wxa8_sb[:, :, g0:g0 + 512],
                              in_=wxa8_r[:, :, g0:g0 + 512])
        # batch inputs: xTsh + all wxb quarters back to back
        nc.sync.dma_start(out=xTsh[:], in_=xT_view(0, SH_COLS))
        wxb_r = wxb.ap().rearrange("(kc p) m -> p kc m", p=128)
        for q in range(4):
            nc.sync.dma_start(out=wx_t[:, :, q * MEM:(q + 1) * MEM],
                              in_=wxb_r[:, :, q * MEM:(q + 1) * MEM])
        nc.sync.dma_start(
            out=wha8_sb[:],
            in_=wha8d.ap().rearrange("p (d m) -> p d m", d=KH8 * 2))
        nc.sync.dma_start(
            out=wfh8_sb[:],
            in_=wfh8d.ap().rearrange("p (d m) -> p d m", d=KH8 * 2))
        nc.sync.dma_start(out=wxa8_sb[:, :, 512:1024],
                          in_=wxa8_r[:, :, 512:1024])
        nc.sync.dma_start(out=whb_sb[:],
                          in_=whb.ap().rearrange("(kc p) m -> p kc m", p=128))
        # all remaining deep-A x chunks, prefetched ahead of the fp16
        # weight streams (buffer-reuse waits may briefly block the queue;
        # everything behind is needed only mid-kernel)
        xt_deep = {}
        for key, c0 in (("l12a", L_OFF[12]), ("c2", L_OFF[13] + 512),
                        ("c3", L_OFF[13] + 768), ("l12b", L_OFF[12] + 256),
                        ("l11", L_OFF[11])):
            # dedicated buffers: these DMAs are emitted before any reader,
            # so pool-buffer reuse would race the earlier tiles' reads
            xt = pAd.tile([128, KC8, 2, 256], H8, tag=f"xtd_{key}", bufs=1,
                          name="xtd")
            nc.sync.dma_start(out=xt[:], in_=xT8_view(c0, 256))
            xt_deep[key] = xt
        nc.sync.dma_start(
            out=whb8_sb[:],
            in_=whb8d.ap().rearrange("p (d m) -> p d m", d=KH8 * 2))

        # B-chain emitter: one item per call, sprinkled between A-deep units
        b_state = {"h": None, "c": None, "idx": 0}

        def emit_b_item():
            i = b_state["idx"]
            b_state["idx"] += 1
            if i > 10:
                return
            if i == 0:
                b_state["h"], b_state["c"] = phaseB_level(pBs, 1, 0, None,
                                                          None, root=True)
            elif i in (1, 2):
                m, xo = (2, 1) if i == 1 else (4, 3)
                b_state["h"], b_state["c"] = phaseB_level(
                    pBs, m, xo, b_state["h"], b_state["c"])
            elif i == 3:
                hpar, cpar = emit_b3_parent(b_state["h"], b_state["c"])
                b_state["h"], b_state["c"] = phaseB_level(
                    pBs, 1, L_OFF[3], hpar, cpar, direct=True)
            else:
                b_state["h"], b_state["c"] = phaseB_level(
                    pBs, 2 ** (i - 3), L_OFF[i], b_state["h"], b_state["c"])

        # levels 13+12 fused: leaf chunks are consumed immediately.
        # A-deep h state in fp8 (DoubleRow moving operands for the deep
        # h-recurrence); h11 stays fp16 (consumed by shallow level 10).
        # The first leaf chunk runs BEFORE the phase-B batch so the tensor
        # engine warms up on fp8 work while the batch weights stream in;
        # the B-shallow chain items are then spread into the cells' mc
        # loops so each item's serial epilogue hides under deep matmuls.
        h12, c12 = alloc_hc(hcA, 512, hdt=H8)
        h13c = {}
        c13c = {}
        for c0 in (0, 256):
            h13c[c0] = pAd.tile([128, KCH, 512], H8, tag="h13c", bufs=1,
                                name="h13c")
            c13c[c0] = pAd.tile([128, KCH, 512], F32, tag="c13c", bufs=1,
                                name="c13c")
        csum_cell(pAd, 256, xt_pre[0], None, None, None, 0, h13c[0],
                  c13c[0], 0, leaf=True)
        csum_cell(pAd, 256, xt_pre[1], None, None, None, 0, h13c[0],
                  c13c[0], 256, leaf=True)
        # phase-B xg batch from wxb (tensor queue: after the warmup cells)
        for j in range(16):
            ps = pp2.tile([128, SH_COLS], F32, tag="ps_F", name="psb")
            for kc in range(KCX):
                nc.tensor.matmul(ps[:], wx_t[:, kc, j * 128:(j + 1) * 128],
                                 xTsh[:, kc, :], start=kc == 0,
                                 stop=kc == KCX - 1)
            nc.scalar.activation(out=xgshB[:, j, :], in_=ps[:],
                                 func=AFT.Identity, bias=bb_sb[:, j:j + 1])
        # fp16 weight streams for the mid-kernel A batch + shallow levels:
        # emitted after the batch so the wx_t overwrite orders behind its
        # readers; transfers stream during the fp8 A-deep phase
        wxa_r = wxa.ap().rearrange("(kc p) m -> p kc m", p=128)
        for q in (0, 2, 3, 1):
            nc.sync.dma_start(out=wx_t[:, :, q * MEM:(q + 1) * MEM],
                              in_=wxa_r[:, :, q * MEM:(q + 1) * MEM])
        nc.sync.dma_start(out=wha_sb[:],
                          in_=wha.ap().rearrange("(kc p) m -> p kc m", p=128))
        nc.sync.dma_start(out=wfh_sb[:],
                          in_=wfh.ap().rearrange("(kc p) m -> p kc m", p=128))
        hs12 = pAd.tile([128, KCH, 256], H8, tag="hs12", bufs=1)
        he, ho = _pairs(h13c[0][:])
        nc.vector.tensor_add(hs12[:], he, ho)
        csum_cell(pAd, 256, xt_deep["l12a"], hs12, h13c[0], c13c[0], 0,
                  h12, c12, 0, leaf=False, emit_cb=emit_b_item)
        csum_cell(pAd, 256, xt_deep["c2"], None, None, None, 0, h13c[256],
                  c13c[256], 0, leaf=True, emit_cb=emit_b_item)
        csum_cell(pAd, 256, xt_deep["c3"], None, None, None, 0, h13c[256],
                  c13c[256], 256, leaf=True, emit_cb=emit_b_item)
        hs12b = pAd.tile([128, KCH, 256], H8, tag="hs12", bufs=1)
        he, ho = _pairs(h13c[256][:])
        nc.vector.tensor_add(hs12b[:], he, ho)
        csum_cell(pAd, 256, xt_deep["l12b"], hs12b, h13c[256], c13c[256],
                  0, h12, c12, 256, leaf=False, emit_cb=emit_b_item)
        # level 11 (h11 output in fp16 for the shallow levels)
        h11, c11 = alloc_hc(hcA, 256)
        hs11 = pAd.tile([128, KCH, 256], H8, tag="hs12", bufs=1)
        he, ho = _pairs(h12[:])
        nc.vector.tensor_add(hs11[:], he, ho)
        csum_cell(pAd, 256, xt_deep["l11"], hs11, h12, c12, 0, h11, c11,
                  0, emit_cb=emit_b_item)
        while b_state["idx"] <= 10:
            emit_b_item()
        # fp8 copy of the level-10 chain h for the B-deep l11 recurrence
        h10b8 = hcB.tile([128, KCH, 128], H8, tag="h10b8", bufs=1)
        nc.vector.tensor_copy(out=h10b8[:], in_=b_state["h"][:, :, 0:128])
        close_pool("pAdeep")

        # xg batch for shallow cols (phase A), then overwrite wx with wxb
        pBd = open_pool("pBdeep", bufs=2)
        pBatchA = open_pool("pBatchA", bufs=2)
        xgshA = pBatchA.tile([128, 16, SH_COLS], H, tag="xgsh", bufs=1)
        for j in range(16):
            blk = WXA_BLK[j]
            ps = pp2.tile([128, SH_COLS], F32, tag="ps_F", name="psb")
            for kc in range(KCX):
                nc.tensor.matmul(ps[:], wx_t[:, kc, blk * 128:(blk + 1) * 128],
                                 xTsh[:, kc, :], start=kc == 0,
                                 stop=kc == KCX - 1)
            nc.scalar.activation(out=xgshA[:, j, :], in_=ps[:],
                                 func=AFT.Identity, bias=ba_sb[:, j:j + 1])
        nc.vector.tensor_copy(out=xgtop[:], in_=xgshA[:, :, 0:7])
        wxb_r2 = wxb.ap().rearrange("(kc p) m -> p kc m", p=128)
        for q in range(4):
            nc.sync.dma_start(out=wx_t[:, :, q * MEM:(q + 1) * MEM],
                              in_=wxb_r2[:, :, q * MEM:(q + 1) * MEM])

        # ---- B-deep emitters (interleaved with A-shallow below) ----
        bd = {"h": b_state["h"], "c": b_state["c"], "xt": None}

        def bdeep_load_xt(l, c0, W):
            xt0 = pBd.tile([128, KCX // 2, W], H, tag="xtB", bufs=4,
                           name="xt0")
            xt1 = pBd.tile([128, KCX // 2, W], H, tag="xtB", bufs=4,
                           name="xt1")
            nc.sync.dma_start(
                out=xt0[:], in_=xT_view(L_OFF[l] + c0, W)[:, 0:KCX // 2, :])
            nc.sync.dma_start(
                out=xt1[:], in_=xT_view(L_OFF[l] + c0, W)[:, KCX // 2:KCX, :])
            return (xt0, xt1)

        def bdeep_gates(l, c0, mc, W, h_pv8, c_pv, xt_pair, gates, st,
                        off=0):
            xt0, xt1 = xt_pair
            p0 = c0 // 2
            hp_v = _drv(h_pv8[:])
            if "pre3" not in st:
                st["pre3"] = pBd.tile([128, 3, W], H, tag="pre3D", bufs=2,
                                      name="pre3")
                st["preu"] = pBd.tile([128, W], H, tag="preD", bufs=2,
                                      name="preu")
            pre3, preu = st["pre3"], st["preu"]
            for gn, gi, j in gates:
                psx = pp4.tile([128, W], F32, tag="ps_g", name="psx")
                for kc in range(KCX):
                    xt_sl = (xt0[:, kc, off:off + W] if kc < KCX // 2
                             else xt1[:, kc - KCX // 2, off:off + W])
                    nc.tensor.matmul(
                        psx[:], wx_t[:, kc, j * 128:(j + 1) * 128],
                        xt_sl, start=kc == 0, stop=kc == KCX - 1)
                psh = pp4.tile([128, W // 2], F32, tag="ps_g", name="psh")
                for dd in range(KH8):
                    nc.tensor.matmul(
                        psh[:], whb8_v[:, dd, :, j * 128:(j + 1) * 128],
                        hp_v[:, dd, :, p0:p0 + W // 2], start=dd == 0,
                        stop=dd == KH8 - 1, perf_mode=DR)
                hp = pBd.tile([128, W // 2], H, tag="hpD", bufs=2,
                              name="hp")
                nc.scalar.activation(out=hp[:], in_=psh[:],
                                     func=AFT.Identity, scale=ISW)
                dst = preu[:] if gi == "u" else pre3[:, gn, :]
                nc.vector.scalar_tensor_tensor(
                    dst.rearrange("p (t two) -> p t two", two=2),
                    psx[:].rearrange("p (t two) -> p t two", two=2),
                    bb_sb[:, j:j + 1], _bcast2(hp[:]),
                    op0=ALU.add, op1=ALU.add)

        def bdeep_tail(l, c0, mc, W, hb, cb, c_pv, st):
            last = l == 13
            p0 = c0 // 2
            pre3, preu = st.pop("pre3"), st.pop("preu")
            g3 = pBd.tile([128, 3, W], H, tag="g3D", bufs=2, name="g3")
            nc.scalar.activation(out=g3[:], in_=pre3[:], func=AFT.Sigmoid)
            gu = pBd.tile([128, W], H, tag="gD_u", bufs=2, name="gu")
            nc.scalar.activation(out=gu[:], in_=preu[:], func=AFT.Tanh)
            if last:
                cn = pBd.tile([128, W], F32, tag="cnD", bufs=1, name="cn")
                c_dst = cn[:]
            else:
                c_dst = cb[:, mc, c0:c0 + W]
            nc.vector.tensor_mul(c_dst, g3[:, 0, :], gu[:])
            pc = c_pv[:, mc, p0:p0 + W // 2]
            t1 = pBd.tile([128, W], F32, tag="t1D", bufs=1, name="t1")
            nc.vector.tensor_mul(
                t1[:].rearrange("p (t two) -> p t two", two=2),
                g3[:, 2, :].rearrange("p (t two) -> p t two", two=2),
                _bcast2(pc))
            nc.vector.tensor_add(c_dst, c_dst, t1[:])
            th = pBd.tile([128, W], H, tag="thD", bufs=2, name="th")
            nc.scalar.activation(out=th[:], in_=c_dst, func=AFT.Tanh)
            if last:
                hm = pBd.tile([128, W], H, tag="hD", bufs=2, name="hm")
                nc.vector.tensor_mul(hm[:], g3[:, 1, :], th[:])
                chain_update(pBd, hm[:], 2 ** (l - 3), mc=mc)
            else:
                nc.vector.tensor_mul(hb[:, mc, c0:c0 + W], g3[:, 1, :], th[:])

        def bdeep_mc(l, c0, mc, W, hb, cb, h_pv8, c_pv, xt_pair, off=0):
            st = {}
            bdeep_gates(l, c0, mc, W, h_pv8, c_pv, xt_pair,
                        ((0, "i", mc), (1, "o", 4 + mc), (2, "f", 8 + mc),
                         (3, "u", 12 + mc)), st, off=off)
            bdeep_tail(l, c0, mc, W, hb, cb, c_pv, st)

        # ---- interleave: A-shallow levels zipped with B-deep l11/l12;
        # the l13 input loads are prefetched near the end of the zip ----
        pAs = open_pool("pAsh", bufs=2)
        hA, cA = h11, c11
        h11b, c11b = alloc_hc(hcB, 256)
        h12b, c12b = alloc_hc(hcB, 512)
        xt13 = {}
        bst = {}
        for i, al in enumerate((10, 9, 8, 7, 6, 5, 4, 3)):
            hA, cA = phaseA_level(pAs, hcA, 2 ** (al - 3), L_OFF[al],
                                  xgshA, hA, cA, wha_sb, wfh_sb)
            if i == 0:
                bd["xt"] = bdeep_load_xt(11, 0, 256)
            mc, half = i // 2, i % 2
            gsel = (((0, "i", mc), (1, "o", 4 + mc)) if half == 0 else
                    ((2, "f", 8 + mc), (3, "u", 12 + mc)))
            bdeep_gates(11, 0, mc, 256, h10b8, bd["c"], bd["xt"], gsel, bst)
            if half == 1:
                bdeep_tail(11, 0, mc, 256, h11b, c11b, bd["c"], bst)
        # level-3 state into persistent staging; collective kicks off NOW,
        # ~45us before anything consumes it, so no engine queue ever blocks
        # on it (the scheduler freely hoists the readback-dependent ops).
        # All collective-adjacent DMAs ride the gpsimd queue.
        nc.vector.tensor_copy(out=h3f[:], in_=hA[:, :, 0:1])
        nc.vector.tensor_copy(out=c3f[:], in_=cA[:, :, 0:1])
        nc.gpsimd.dma_start(
            out=cc_in.ap()[0:MEM].rearrange("(p q) -> p q", p=128),
            in_=h3f[:, :, 0])
        nc.gpsimd.dma_start(
            out=cc_in.ap()[MEM:2 * MEM].rearrange("(p q) -> p q", p=128),
            in_=c3f[:, :, 0])
        nc.gpsimd.collective_compute(
            "AllGather", ALU.bypass,
            replica_groups=[list(range(NCORES))],
            ins=[cc_in.ap()], outs=[cc_out.ap()],
        )
        chain_update(pBd, h11b[:, :, 0:256], 256)
        h11b8 = hcB.tile([128, KCH, 256], H8, tag="h11b8", bufs=1)
        nc.vector.tensor_copy(out=h11b8[:], in_=h11b[:, :, 0:256])
        for v in range(4):
            if v == 0:
                bd["xt"] = bdeep_load_xt(12, 0, 512)
            bdeep_mc(12, 0, v, 512, h12b, c12b, h11b8, c11b, bd["xt"])
            if v == 0:
                xt13[0] = bdeep_load_xt(13, 0, 512)
            if v == 3:
                xt13[512] = bdeep_load_xt(13, 512, 512)
                chain_update(pBd, h12b[:, :, 0:512], 512)
        h12b8 = hcB.tile([128, KCH, 512], H8, tag="h12b8", bufs=1)
        nc.vector.tensor_copy(out=h12b8[:], in_=h12b[:, :, 0:512])

        h_pv, c_pv = b_state["h"], b_state["c"]
        close_pool("pAsh")
        close_pool("pBatchA")

        # ---- B-deep level 13 (storeless) + phase-A top. The collective is
        # long done by now; the A-top levels interleave with the second l13
        # chunk so their serial epilogues hide under the big matmul streams.
        for mc in range(4):
            bdeep_mc(13, 0, mc, 512, None, None, h12b8, c12b, xt13[0])
        nc.gpsimd.dma_start(
            out=h3f32[:],
            in_=cc_out.ap()[:, 0:MEM].rearrange("n (p q) -> p n q", p=128))
        nc.gpsimd.dma_start(
            out=c3all[:],
            in_=cc_out.ap()[:, MEM:2 * MEM].rearrange("n (p q) -> p n q",
                                                      p=128))
        nc.vector.tensor_copy(out=h3all[:], in_=h3f32[:])
        hT_pv = h3all[:].rearrange("p n q -> p q n")
        cT_pv = c3all[:].rearrange("p n q -> p q n")
        atop = ((2, 3), (1, 1), (0, 0))
        for mc in range(3):
            bdeep_mc(13, 512, mc, 512, None, None, h12b8, c12b, xt13[512])
            tl, xo = atop[mc]
            hT_pv, cT_pv = phaseA_level(pT, hcT, 2 ** tl, xo, xgtop,
                                        hT_pv, cT_pv, wha_sb, wfh_sb)
        bdeep_mc(13, 512, 3, 256, None, None, h12b8, c12b, xt13[512])
        bdeep_mc(13, 768, 3, 256, None, None, h12b8, c12b, xt13[512],
                 off=256)
        h0f = pT.tile([128, KCH, 1], F32, tag="h0f", bufs=1)
        nc.vector.tensor_copy(out=h0f[:], in_=hT_pv[:, :, 0:1])
        nc.sync.dma_start(
            out=out.ap()[0, 0:MEM].rearrange("(q p) -> p q", p=128),
            in_=h0f[:, :, 0])
        close_pool("pBdeep")
        close_pool("hcA")
        close_pool("pBsh")
        close_pool("hcB")

        # final chain-max output
        nc.sync.dma_start(
            out=out.ap()[0, MEM:2 * MEM].rearrange("(q p) -> p q", p=128),
            in_=cmax[:, 0:KCH])

        close_pool("hcT")
        close_pool("pTop")
        close_pool("pp2")
        close_pool("pp4")
        close_pool("persist")

    nc.compile()
    return nc


def _q8(a, s):
    """float -> TRN fp8e4 (== ml_dtypes.float8_e4m3, inf at 256) with scale"""
    v = np.clip(np.asarray(a, np.float32) * s, -240.0, 240.0)
    return v.astype(ml_dtypes.float8_e4m3)


def _pack8_x(wT):
    """[IN, n] -> [128, KC8*2*n] with contraction row = q*256 + i*128 + p"""
    n = wT.shape[1]
    return np.ascontiguousarray(
        wT.reshape(KC8, 2, 128, n).transpose(2, 0, 1, 3).reshape(128, -1))


def _pack8_h(wT):
    """[MEM, n] -> [128, KH8*2*n] with contraction row = d*256 + i*128 + p"""
    n = wT.shape[1]
    return np.ascontiguousarray(
        wT.reshape(KH8, 2, 128, n).transpose(2, 0, 1, 3).reshape(128, -1))


def _host_inputs(inputs, ifoux_w, ifoux_b, ious_w, ious_b, fh_w, fh_b,
                 iofux_w, iofux_b, iofuh_w, iofuh_b):
    """Build the 8 per-core input maps (host-side sharding / layout only)."""
    f32 = np.float32
    inputs = np.asarray(inputs, f32)
    m = MEM

    f16 = np.float16
    wxa_t = np.ascontiguousarray(np.asarray(ifoux_w, f32).T)
    wha_t = np.ascontiguousarray(np.asarray(ious_w, f32).T)
    wfh_t = np.ascontiguousarray(np.asarray(fh_w, f32).T)
    wxa = wxa_t.astype(f16)
    wha = wha_t.astype(f16)
    wfhT = wfh_t.astype(f16)
    wxb = np.ascontiguousarray(np.asarray(iofux_w, f32).T.astype(f16))
    whb = np.ascontiguousarray(np.asarray(iofuh_w, f32).T.astype(f16))
    wxa8 = _pack8_x(_q8(wxa_t, SW))
    wha8 = _pack8_h(_q8(wha_t, SW))
    wfh8 = _pack8_h(_q8(wfh_t, SW))
    whb8 = _pack8_h(_q8(np.asarray(iofuh_w, f32).T, SW))

    ifoux_b = np.asarray(ifoux_b, f32)
    ious_b = np.asarray(ious_b, f32)
    fh_b = np.asarray(fh_b, f32)
    # phase-A folded biases in j-order i,o,u,f
    ba = np.concatenate([
        ifoux_b[0:m] + ious_b[0:m],                  # i
        ifoux_b[2 * m:3 * m] + ious_b[m:2 * m],      # o
        ifoux_b[3 * m:4 * m] + ious_b[2 * m:3 * m],  # u
        ifoux_b[m:2 * m] + fh_b,                     # f (+ fh bias)
    ])
    ba = np.ascontiguousarray(ba.reshape(16, 128).T)
    bb = np.asarray(iofux_b, f32) + np.asarray(iofuh_b, f32)
    bb = np.ascontiguousarray(bb.reshape(16, 128).T)

    in_maps = []
    for k in range(NCORES):
        idx = [np.arange(7)]
        for l in range(3, DEPTH):
            w = 2 ** (l - 3)
            idx.append((2 ** l - 1) + k * w + np.arange(w))
        idx = np.concatenate(idx)
        xkT = inputs[idx].T  # [IN, NCOLS] fp32
        xk = np.ascontiguousarray(xkT.astype(f16))
        xk8 = _pack8_x(_q8(xkT[:, SH_COLS:], 1.0))
        sel = np.zeros((128, 4), f32)
        sel[:, k // 2] = 1.0
        in_maps.append({
            "xT": xk, "xT8": xk8, "wxa": wxa, "wxa8": wxa8, "wha": wha,
            "wha8": wha8, "wfh": wfhT, "wfh8": wfh8, "wxb": wxb,
            "whb": whb, "whb8": whb8, "ba": ba, "bb": bb, "sel3": sel,
        })
    return in_maps


def _get_prog():
    global _PROG
    if _PROG is None:
        _PROG = build()
    return _PROG


def kernel(inputs, ifoux_w, ifoux_b, ious_w, ious_b, fh_w, fh_b,
           iofux_w, iofux_b, iofuh_w, iofuh_b, depth=DEPTH, **_unused):
    assert int(depth) == DEPTH, f"kernel hardcodes depth={DEPTH}"
    nc = _get_prog()
    in_maps = _host_inputs(inputs, ifoux_w, ifoux_b, ious_w, ious_b,
                           fh_w, fh_b, iofux_w, iofux_b, iofuh_w, iofuh_b)
    res = run_bass_kernel_spmd(nc, in_maps, list(range(NCORES)))
    outs = [res.results[k]["out"][0] for k in range(NCORES)]
    root = outs[0][:MEM]
    cmax = np.max(np.stack([o[MEM:] for o in outs]), axis=0)
    return np.concatenate([root, cmax])[None, :].astype(np.float32)


if __name__ == "__main__":
    import sys
    if len(sys.argv) > 1 and sys.argv[1] == "emit":
        real_compile = bacc.Bacc.compile
        bacc.Bacc.compile = lambda self: None
        try:
            build()
            print("emit OK")
        finally:
            bacc.Bacc.compile = real_compile
